# revision 8
# baseline (speedup 1.0000x reference)
"""CocktailGNN (3-layer GraphSAGE, mean aggregation) on 8 trn2 NeuronCores.

Strategy v2: shard by destination-node range (NPC = N/8 = 6250 per core).
Layer-1 input transform folded into host prep; ships as a replicated fp8
node table t0 plus a per-core f16 h1T for the self (Wr) path.

Per layer on device:
  - ONE fp8 node table [50176, HID] in DRAM per layer boundary, built by a
    single AllGather of each core's rows (one big collective gets the best
    effective bandwidth; two int16-addressable halves via base-offset APs
    keep dma_gather happy: half A = rows < 25088, half B = the rest);
  - table rows live in a PERMUTED layout: node (core c, local l) at row
    c*6272 + (l%128)*49 + l//128, which makes the per-core rows buffer a
    partition-major [128, 49*HID] image -> phase_c stages all transposed
    blocks into one SBUF tile and writes DRAM with a single contiguous DMA;
  - neighbor rows gathered fp8 via indirect DMA (<=512 idx per call);
  - segment-sum via one-hot matmul on TensorE in fp8 DoubleRow perf mode
    (two 128-slot chunks per instruction, 2x rate), accumulated across both
    halves in one PSUM pass, then scaled by inv-degree (DVE mul);
  - SAGE linears as dense f16 matmuls with fused bias+relu on ScalarE in
    [F, nodes] layout; transpose hT via PE for the next boundary.
"""

import os
import sys

sys.path.insert(0, "/opt/trn_rl_repo")
os.environ.setdefault("MYCRO_LOCAL_CACHE", "1")

import numpy as np

import concourse.bass as bass
import concourse.bacc as bacc
import concourse.mybir as mybir
import concourse.tile as tile
from concourse import bass_utils
from concourse.masks import make_identity

P = 128
F32 = mybir.dt.float32
F16 = mybir.dt.float16
FP8 = mybir.dt.float8e4
I16 = mybir.dt.int16
I32 = mybir.dt.int32
NP_FP8 = mybir.dt.np(FP8)

# >512 idxs per dma_gather hangs real HW (ucode SWDGE ring is fixed-size
# regardless of the build-time carveout), so cap calls at 4 chunks of 128.
DMA_SCRATCH = int(os.environ.get("GNN_DMA_SCRATCH", "16384"))
GC = int(os.environ.get("GNN_GC", "4"))  # max one-hot chunks per dma_gather
NQ = int(os.environ.get("GNN_QUEUES", "4"))  # SWDGE queues to round-robin
TBL16 = os.environ.get("GNN_TBL16", "0") == "1"  # f16 layer-2/3 tables


class Cfg:
    def __init__(self, N=50000, E=800000, in_dim=2, emb=128, hid=256, r=8):
        assert N % r == 0
        self.N, self.E, self.IN_DIM, self.EMB, self.HID, self.R = N, E, in_dim, emb, hid, r
        self.NPC = N // r                      # nodes per core
        self.B = (self.NPC + P - 1) // P       # 128-node blocks per core
        self.NPB = self.B * P                  # padded nodes per core
        self.TBL = r * self.NPB                # 50176 table rows
        self.HA = (r // 2) * self.NPB          # 25088 <= 32768 (int16 ok)
        self.HB = self.TBL - self.HA           # 25088


def host_prep(cfg: Cfg, x, edge_index, W_in, b_in, layers):
    """Build per-core input maps. layers = [(Wl, bl, Wr)] * 3.

    Edges bucketed by (core, dst-block, src-half). Table row for src node
    (core c, local l) = c*NPB + (l%128)*B + l//128  (permuted layout that
    matches the partition-major rows image each core stages). Half A:
    row < HA (cores 0-3); half B: the rest (idx = row - HA). Each
    (block, half) padded to C_A/C_B chunks of 128 slots; pad slots gather
    row 0 with d_rel = -1.
    """
    src = np.asarray(edge_index[0], dtype=np.int64)
    dst = np.asarray(edge_index[1], dtype=np.int64)
    N, E, R, B, NPC, NPB = cfg.N, cfg.E, cfg.R, cfg.B, cfg.NPC, cfg.NPB
    HA = cfg.HA

    deg = np.bincount(dst, minlength=N).astype(np.float64)
    inv = np.where(deg > 0, 1.0 / np.maximum(deg, 1), 0.0).astype(np.float32)

    core = dst // NPC
    local = dst % NPC
    blk = local // P
    drel = (local % P).astype(np.float32)
    group = core * B + blk

    s_core = src // NPC
    s_loc = src % NPC
    srow = s_core * NPB + (s_loc % P) * B + s_loc // P
    regB = (srow >= HA).astype(np.int64)
    srow16 = srow - regB * HA
    grp2 = group * 2 + regB

    cnt2 = np.bincount(grp2, minlength=R * B * 2)
    C_A = int(np.ceil(cnt2[0::2].max() / P))
    C_B = int(np.ceil(cnt2[1::2].max() / P))
    S_A, S_B = C_A * P, C_B * P

    order = np.argsort(grp2, kind="stable")
    offs = np.zeros(R * B * 2 + 1, np.int64)
    np.cumsum(cnt2, out=offs[1:])
    pos = np.arange(E) - offs[grp2[order]]
    gs = group[order]
    regs = regB[order].astype(bool)
    srow_s = srow16[order]
    drel_s = drel[order]

    mA, mB = ~regs, regs
    flatA = gs[mA] * S_A + pos[mA]
    flatB = gs[mB] * S_B + pos[mB]

    idxA = np.zeros(R * B * S_A, np.int16)
    idxA[flatA] = srow_s[mA].astype(np.int16)
    idxB = np.zeros(R * B * S_B, np.int16)
    idxB[flatB] = srow_s[mB].astype(np.int16)

    drelA = np.full(R * B * S_A, -1.0, np.float32)
    drelA[flatA] = drel_s[mA]
    drelB = np.full(R * B * S_B, -1.0, np.float32)
    drelB[flatB] = drel_s[mB]

    # d_rel device layout: [128, B*CT], col b*CT + c, partition = slot % 128
    CT = C_A + C_B
    dA = drelA.reshape(R, B, C_A, P)
    dB = drelB.reshape(R, B, C_B, P)
    drel_dev = np.concatenate([dA, dB], axis=2).transpose(0, 3, 1, 2).reshape(
        R, P, B * CT).astype(np.float16)

    # idx device layout: index i of a half lives at [i % 16, i // 16],
    # replicated across the 8 groups of 16 partitions.
    def idx_dev(arr, S):
        a = arr.reshape(R, B, S // 16, 16).transpose(0, 3, 1, 2).reshape(R, 16, B * (S // 16))
        return np.ascontiguousarray(np.tile(a, (1, 8, 1)))

    idxA_dev = idx_dev(idxA, S_A)
    idxB_dev = idx_dev(idxB, S_B)

    invrow = np.zeros((R, 1, NPB), np.float32)
    invrow[:, 0, :NPC] = inv.reshape(R, NPC)

    # layer-1 node table (host-computed input transform), permuted fp8 rows
    h1 = np.maximum(np.asarray(x, np.float32) @ np.asarray(W_in, np.float32)
                    + np.asarray(b_in, np.float32), 0.0)          # [N, EMB]
    EMB = cfg.EMB
    h1c = h1.reshape(R, NPC, EMB)
    # layer-1 table stays f16: dma_gather needs 256B-multiple rows, and
    # 128 fp8 features is only 128B.
    t0 = np.zeros((cfg.TBL, EMB), np.float16)
    l = np.arange(NPC)
    prow = (l % P) * B + l // P
    t0.reshape(R, NPB, EMB)[:, prow] = h1c

    h1T = np.zeros((R, EMB, NPB), np.float16)
    h1T[:, :, :NPC] = h1c.transpose(0, 2, 1)

    in_maps = []
    for r in range(R):
        m = {
            "h1T": h1T[r],
            "invrow": invrow[r],
            "idxA": idxA_dev[r],
            "idxB": idxB_dev[r],
            "drel": np.ascontiguousarray(drel_dev[r]),
            "t0": t0,   # replicated
        }
        for li, (Wl, bl, Wr) in enumerate(layers, start=1):
            m[f"wl{li}"] = np.asarray(Wl, np.float16)
            m[f"bl{li}"] = np.asarray(bl, np.float32)
            m[f"wr{li}"] = np.asarray(Wr, np.float16)
        in_maps.append(m)
    return in_maps, (C_A, C_B)


def emit(tc: tile.TileContext, outs, ins, cfg: Cfg, C, repeat=1, only=None):
    """only: None for the full pipeline, else a set of phase tags among
    {"a","b","c","ag","a_gather","a_m","a_mm"} for timing ablations."""
    keep = (lambda t: True) if only is None else (lambda t: t in only)
    qrr = [0]
    C_A, C_B = C
    CT = C_A + C_B
    nc = tc.nc
    B, NPB = cfg.B, cfg.NPB
    EMB, HID, R = cfg.EMB, cfg.HID, cfg.R
    groups = [list(range(R))]
    f = F32
    DR = mybir.MatmulPerfMode.DoubleRow
    TDT = F16 if TBL16 else FP8

    # node chunks for the dense-linear phase
    t_chunks = []
    t0c = 0
    while t0c < NPB:
        w = min(512, NPB - t0c)
        t_chunks.append((t0c, w))
        t0c += w

    from contextlib import ExitStack
    ctx = ExitStack()
    const = ctx.enter_context(tc.tile_pool(name="const", bufs=1))
    work = ctx.enter_context(tc.tile_pool(name="work", bufs=int(os.environ.get("GNN_WORK_BUFS", "2"))))
    psA = ctx.enter_context(tc.tile_pool(name="psA", bufs=1, space="PSUM"))
    dram = ctx.enter_context(tc.tile_pool(name="dram", bufs=1, space="DRAM"))

    # ---------------- persistent SBUF ----------------
    h1T = const.tile([P, NPB], F16, name="h1T")
    hT = [const.tile([P, NPB], F16, name=f"hT{k}") for k in range(HID // P)]
    aggT = [const.tile([P, NPB], F16, name=f"aggT{k}") for k in range(HID // P)]
    invrep = const.tile([P, NPB], F16, name="invrep")
    iota_i = const.tile([P, P], I32, name="iota_i")
    iota_h = const.tile([P, P], F16, name="iota_h")
    ident = const.tile([P, P], F16, name="ident")
    idxA_sb = const.tile([P, B * (C_A * P // 16)], I16, name="idxA_sb")
    idxB_sb = const.tile([P, B * (C_B * P // 16)], I16, name="idxB_sb")
    drel_h = const.tile([P, B * CT], F16, name="drel_h")
    drel_f = const.tile([P, B * CT], F32, name="drel_f")
    ones_sb = const.tile([1, P], F16, name="ones_sb")
    rows_sb = const.tile([P, B * HID], TDT, name="rows_sb")

    wl_sb, wr_sb, bl_sb, f_ins = {}, {}, {}, {}
    for li in range(1, 4):
        fin = EMB if li == 1 else HID
        f_ins[li] = fin
        wl_sb[li] = [const.tile([P, HID], F16, name=f"wl{li}_{k}") for k in range(fin // P)]
        wr_sb[li] = [const.tile([P, HID], F16, name=f"wr{li}_{k}") for k in range(fin // P)]
        bl_sb[li] = const.tile([P, HID // P], f, name=f"bl{li}_sb")

    # ---------------- DRAM tables & bounce buffers (per iteration) --------
    def make_tables(it):
        sfx = f"_i{it}" if it else ""
        t = {}
        for li in (1, 2):
            t[li] = (
                dram.tile([cfg.TBL, HID], TDT, addr_space="Shared", name=f"tbl{li}{sfx}"),
                dram.tile([P, B * HID], TDT, name=f"rows{li}{sfx}"),
            )
        return t

    # ---------------- load constants ----------------
    nc.sync.dma_start(h1T[:], ins["h1T"][:])
    nc.sync.dma_start(idxA_sb[:], ins["idxA"][:])
    nc.sync.dma_start(idxB_sb[:], ins["idxB"][:])
    nc.sync.dma_start(drel_h[:], ins["drel"][:])
    nc.vector.tensor_copy(drel_f[:], drel_h[:])
    for li in range(1, 4):
        fin = f_ins[li]
        for k in range(fin // P):
            nc.sync.dma_start(wl_sb[li][k][:], ins[f"wl{li}"][k * P:(k + 1) * P, :])
            nc.sync.dma_start(wr_sb[li][k][:], ins[f"wr{li}"][k * P:(k + 1) * P, :])
        for j in range(HID // P):
            nc.sync.dma_start(bl_sb[li][:, j:j + 1], ins[f"bl{li}"][j * P:(j + 1) * P, None])

    nc.gpsimd.iota(iota_i[:], pattern=[[1, P]], base=0, channel_multiplier=0)
    nc.vector.tensor_copy(iota_h[:], iota_i[:])
    nc.vector.memset(ones_sb[:], 1.0)
    make_identity(nc, ident[:])

    if only is not None:  # ablation mode: make all persistent tiles readable
        for t in hT + aggT:
            nc.vector.memset(t[:], 0.0)
        nc.vector.memset(rows_sb[:], 0.0)

    # invrep[p, n] = inv_deg[n] for all p, via rank-1 matmul broadcast
    for (ts, w) in t_chunks:
        invrow_sb = work.tile([1, 512], F16, tag="invrow", name="invrow_sb")
        nc.gpsimd.dma_start(invrow_sb[:, :w], ins["invrow"][:, ts:ts + w])
        pb = psA.tile([P, 512], f, tag="hlin", bufs=2, name="pb_inv")
        nc.tensor.matmul(pb[:, :w], lhsT=ones_sb[:, :], rhs=invrow_sb[:, :w],
                         start=True, stop=True)
        nc.scalar.copy(invrep[:, ts:ts + w], pb[:, :w])

    def phase_a(fin, tblA, tblB, dt=FP8):
        """Gather rows + one-hot segment-sum into aggT (DoubleRow when fp8)."""
        nk = fin // P
        wIA = C_A * P // 16
        wIB = C_B * P // 16

        gfix = None
        if not keep("a_gather"):  # ablation: stable zero G
            key = (CT, fin, dt)
            if key not in gfix_cache:
                t = const.tile([P, CT, fin], dt, name=f"Gfix{len(gfix_cache)}")
                nc.vector.memset(t[:], 0.0)
                gfix_cache[key] = t
            gfix = gfix_cache[key]
        for b in range(B):
            G = work.tile([P, CT, fin], dt, tag="G", name="G") \
                if gfix is None else gfix
            if keep("a_gather"):
                for (tbl, c0, cN, idx_sb, wI) in (
                        (tblA, 0, C_A, idxA_sb, wIA),
                        (tblB, C_A, C_B, idxB_sb, wIB)):
                    g0 = 0
                    while g0 < cN:
                        g1 = min(g0 + GC, cN)
                        n = (g1 - g0) * P
                        nc.gpsimd.dma_gather(
                            G[:, c0 + g0:c0 + g1, :], tbl,
                            idx_sb[:, b * wI + g0 * 8: b * wI + g1 * 8],
                            n, n, fin, queue_num=qrr[0] % NQ)
                        qrr[0] += 1
                        g0 = g1
            M = work.tile([P, CT, P], dt, tag="M", name="M")
            if keep("a_m"):
                for c in range(CT):
                    col = b * CT + c
                    nc.vector.tensor_scalar(
                        out=M[:, c, :], in0=iota_h[:],
                        scalar1=drel_f[:, col:col + 1], scalar2=None,
                        op0=mybir.AluOpType.is_equal)
            if keep("a_mm"):
                ps = [psA.tile([P, P], f, tag=f"agg{k}", bufs=2,
                               name=f"aggp{k}") for k in range(nk)]
                for k in range(nk):
                    c = 0
                    while c < CT:
                        if dt == FP8 and c + 1 < CT:
                            nc.tensor.matmul(ps[k][:],
                                             lhsT=G[:, c:c + 2, k * P:(k + 1) * P],
                                             rhs=M[:, c:c + 2, :],
                                             start=(c == 0), stop=(c + 2 >= CT),
                                             perf_mode=DR)
                            c += 2
                        else:
                            nc.tensor.matmul(ps[k][:],
                                             lhsT=G[:, c, k * P:(k + 1) * P],
                                             rhs=M[:, c, :],
                                             start=(c == 0), stop=(c + 1 >= CT))
                            c += 1
                bl = slice(b * P, (b + 1) * P)
                for k in range(nk):
                    nc.vector.tensor_mul(aggT[k][:, bl], ps[k][:], invrep[:, bl])

    def phase_b(li):
        fin = f_ins[li]
        nk = fin // P
        hin = [h1T] if li == 1 else hT
        for (ts, w) in t_chunks:
            ph = [psA.tile([P, 512], f, tag="hlin", bufs=2, name=f"ph{li}_{j}")
                  for j in range(HID // P)]
            for j in range(HID // P):
                n_mm = 2 * nk
                i_mm = 0
                for k in range(nk):
                    nc.tensor.matmul(ph[j][:, :w], lhsT=wl_sb[li][k][:, j * P:(j + 1) * P],
                                     rhs=aggT[k][:, ts:ts + w],
                                     start=(i_mm == 0), stop=(i_mm == n_mm - 1))
                    i_mm += 1
                for k in range(nk):
                    nc.tensor.matmul(ph[j][:, :w], lhsT=wr_sb[li][k][:, j * P:(j + 1) * P],
                                     rhs=hin[k][:, ts:ts + w],
                                     start=(i_mm == 0), stop=(i_mm == n_mm - 1))
                    i_mm += 1
            for j in range(HID // P):
                nc.scalar.activation(hT[j][:, ts:ts + w], ph[j][:, :w],
                                     mybir.ActivationFunctionType.Relu,
                                     bias=bl_sb[li][:, j:j + 1])

    def phase_c(tbl=None, rows=None, rows_out=None):
        """transpose hT -> staged rows; one DMA + one AllGather per boundary.
        If rows_out is given, write f32 node-major rows there (final out)."""
        if not keep("c"):
            return
        if rows_out is not None:
            for b in range(B):
                stage = work.tile([P, HID], F32, tag="stage", name="stage")
                for j in range(HID // P):
                    trp = psA.tile([P, P], F16, tag="trp", bufs=2, name="trp_c")
                    nc.tensor.transpose(trp[:], hT[j][:, b * P:(b + 1) * P], ident[:])
                    nc.vector.tensor_copy(stage[:, j * P:(j + 1) * P], trp[:])
                nc.sync.dma_start(rows_out[b * P:(b + 1) * P, :], stage[:])
            return
        for b in range(B):
            for j in range(HID // P):
                trp = psA.tile([P, P], F16, tag="trp", bufs=2, name="trp_c")
                nc.tensor.transpose(trp[:], hT[j][:, b * P:(b + 1) * P], ident[:])
                nc.vector.tensor_copy(
                    rows_sb[:, b * HID + j * P: b * HID + (j + 1) * P], trp[:])
        nc.sync.dma_start(rows[:, :], rows_sb[:, :])
        if keep("ag"):
            nc.gpsimd.collective_compute(
                "AllGather", mybir.AluOpType.bypass, replica_groups=groups,
                ins=[rows.opt()], outs=[tbl.opt()])

    def pipeline(it):
        t = make_tables(it)
        if keep("a"):
            phase_a(EMB, ins["t0"][:cfg.HA, :], ins["t0"][cfg.HA:, :], dt=F16)
        if keep("b"):
            phase_b(1)
        phase_c(*t[1])
        if keep("a"):
            phase_a(HID, t[1][0][:cfg.HA, :], t[1][0][cfg.HA:, :], dt=TDT)
        if keep("b"):
            phase_b(2)
        phase_c(*t[2])
        if keep("a"):
            phase_a(HID, t[2][0][:cfg.HA, :], t[2][0][cfg.HA:, :], dt=TDT)
        if keep("b"):
            phase_b(3)
        phase_c(rows_out=outs["h_out"])

    gfix_cache = {}
    for it in range(repeat):
        pipeline(it)
    ctx.close()


def build_program(cfg: Cfg, C, repeat=1, only=None):
    C_A, C_B = C
    CT = C_A + C_B
    nc = bacc.Bacc("TRN2", target_bir_lowering=False, debug=False,
                   enable_asserts=True, num_devices=cfg.R,
                   dynamic_dma_scratch_size=DMA_SCRATCH,
                   num_swdge_queues=NQ)
    ins = {
        "h1T": nc.dram_tensor("h1T", [cfg.EMB, cfg.NPB], F16, kind="ExternalInput").ap(),
        "invrow": nc.dram_tensor("invrow", [1, cfg.NPB], F32, kind="ExternalInput").ap(),
        "idxA": nc.dram_tensor("idxA", [P, cfg.B * (C_A * P // 16)], I16, kind="ExternalInput").ap(),
        "idxB": nc.dram_tensor("idxB", [P, cfg.B * (C_B * P // 16)], I16, kind="ExternalInput").ap(),
        "drel": nc.dram_tensor("drel", [P, cfg.B * CT], F16, kind="ExternalInput").ap(),
        "t0": nc.dram_tensor("t0", [cfg.TBL, cfg.EMB], F16, kind="ExternalInput").ap(),
    }
    for li in range(1, 4):
        fin = cfg.EMB if li == 1 else cfg.HID
        ins[f"wl{li}"] = nc.dram_tensor(f"wl{li}", [fin, cfg.HID], F16, kind="ExternalInput").ap()
        ins[f"bl{li}"] = nc.dram_tensor(f"bl{li}", [cfg.HID], F32, kind="ExternalInput").ap()
        ins[f"wr{li}"] = nc.dram_tensor(f"wr{li}", [fin, cfg.HID], F16, kind="ExternalInput").ap()
    outs = {
        "h_out": nc.dram_tensor("h_out", [cfg.NPB, cfg.HID], F32, kind="ExternalOutput").ap(),
    }
    with tile.TileContext(nc) as tc:
        emit(tc, outs, ins, cfg, C, repeat=repeat, only=only)
    nc.compile()
    return nc


def make_runner(nc, in_maps, n_cores):
    """Build a pinned-input PJRT runner: inputs are device_put once, outputs
    are donated back as the next call's (ignored) output buffers, so warm
    calls measure execution + dispatch only."""
    import jax
    from jax.sharding import Mesh, PartitionSpec, NamedSharding
    from jax.experimental.shard_map import shard_map
    from concourse import bass2jax, mybir as mb
    bass2jax.install_neuronx_cc_hook()

    partition_name = nc.partition_id_tensor.name if nc.partition_id_tensor else None
    in_names, out_names, out_avals, zero_outs = [], [], [], []
    for alloc in nc.m.functions[0].allocations:
        if not isinstance(alloc, mb.MemoryLocationSet):
            continue
        name = alloc.memorylocations[0].name
        if alloc.kind == "ExternalInput":
            if name != partition_name:
                in_names.append(name)
        elif alloc.kind == "ExternalOutput":
            shape = tuple(alloc.tensor_shape)
            dtype = mb.dt.np(alloc.dtype)
            out_names.append(name)
            out_avals.append(jax.core.ShapedArray(shape, dtype))
            zero_outs.append(np.zeros(shape, dtype))
    n_params = len(in_names)
    n_outs = len(out_avals)
    all_in_names = list(in_names) + list(out_names)
    if partition_name is not None:
        all_in_names.append(partition_name)
    donate = tuple(range(n_params, n_params + n_outs))

    def _body(*args):
        operands = list(args)
        if partition_name is not None:
            operands.append(bass2jax.partition_id_tensor())
        outs = bass2jax._bass_exec_p.bind(
            *operands,
            out_avals=tuple(out_avals),
            in_names=tuple(all_in_names),
            out_names=tuple(out_names),
            lowering_input_output_aliases=(),
            sim_require_finite=True,
            sim_require_nnan=True,
            nc=nc,
        )
        return tuple(outs)

    devices = jax.devices()[:n_cores]
    mesh = Mesh(np.asarray(devices), ("core",))
    sharded = jax.jit(
        shard_map(_body, mesh=mesh,
                  in_specs=(PartitionSpec("core"),) * (n_params + n_outs),
                  out_specs=(PartitionSpec("core"),) * n_outs,
                  check_rep=False),
        donate_argnums=donate, keep_unused=True)
    sh = NamedSharding(mesh, PartitionSpec("core"))
    concat_in = [np.concatenate([np.asarray(m[nm]) for m in in_maps], axis=0)
                 for nm in in_names]
    d_in = [jax.device_put(a, sh) for a in concat_in]
    state = {"donate": [jax.device_put(
        np.zeros((n_cores * z.shape[0], *z.shape[1:]), z.dtype), sh) for z in zero_outs]}

    def run():
        outs = sharded(*d_in, *state["donate"])
        jax.block_until_ready(outs)
        state["donate"] = list(outs)
        return outs

    def results():
        outs = state["donate"]
        return [{nm: np.asarray(outs[i]).reshape(n_cores, *out_avals[i].shape)[c]
                 for i, nm in enumerate(out_names)} for c in range(n_cores)]

    return run, results


def kernel(**inputs) -> np.ndarray:
    cfg = Cfg()
    layers = [(inputs["Wl1"], inputs["bl1"], inputs["Wr1"]),
              (inputs["Wl2"], inputs["bl2"], inputs["Wr2"]),
              (inputs["Wl3"], inputs["bl3"], inputs["Wr3"])]
    in_maps, C = host_prep(cfg, inputs["x"], inputs["edge_index"],
                           inputs["W_in"], inputs["b_in"], layers)
    nc = build_program(cfg, C)
    # the NTFF trace hook (antenv.axon_hooks) is absent in this container;
    # make sure run_bass_kernel_spmd never takes the trace path.
    os.environ["BASS_NEVER_TRACE"] = "1"
    res = bass_utils.run_bass_kernel_spmd(
        nc, in_maps, core_ids=list(range(cfg.R)), trace=False)
    if res.exec_time_ns is not None:
        print(f"HW exec time: {res.exec_time_ns} ns")
    out = np.concatenate(
        [res.results[r]["h_out"][:cfg.NPC] for r in range(cfg.R)], axis=0)
    return out.astype(np.float32)


if __name__ == "__main__":
    # smoke: build only
    cfg = Cfg()
    rng = np.random.default_rng(0)
    ei = rng.integers(0, cfg.N, size=(2, cfg.E), dtype=np.int64)
    x = rng.standard_normal((cfg.N, cfg.IN_DIM), dtype=np.float32)
    layers = [(rng.standard_normal((cfg.EMB, cfg.HID), dtype=np.float32) * 0.1,
               np.zeros(cfg.HID, np.float32),
               rng.standard_normal((cfg.EMB, cfg.HID), dtype=np.float32) * 0.1)]
    layers += [(rng.standard_normal((cfg.HID, cfg.HID), dtype=np.float32) * 0.1,
                np.zeros(cfg.HID, np.float32),
                rng.standard_normal((cfg.HID, cfg.HID), dtype=np.float32) * 0.1) for _ in range(2)]
    in_maps, C = host_prep(cfg, x, ei, rng.standard_normal((cfg.IN_DIM, cfg.EMB), dtype=np.float32),
                           np.zeros(cfg.EMB, np.float32), layers)
    print("C =", C)
    nc = build_program(cfg, C)
    print("built ok; instructions:",
          sum(len(bb.instructions) for fn in nc.m.functions for bb in fn.blocks))


# revision 10
# speedup vs baseline: 8.0535x; 8.0535x over previous
"""CocktailGNN (3-layer GraphSAGE, mean aggregation) on 8 trn2 NeuronCores.

Strategy v2: shard by destination-node range (NPC = N/8 = 6250 per core).
Layer-1 input transform folded into host prep; ships as a replicated fp8
node table t0 plus a per-core f16 h1T for the self (Wr) path.

Per layer on device:
  - ONE fp8 node table [50176, HID] in DRAM per layer boundary, built by a
    single AllGather of each core's rows (one big collective gets the best
    effective bandwidth; two int16-addressable halves via base-offset APs
    keep dma_gather happy: half A = rows < 25088, half B = the rest);
  - table rows live in a PERMUTED layout: node (core c, local l) at row
    c*6272 + (l%128)*49 + l//128, which makes the per-core rows buffer a
    partition-major [128, 49*HID] image -> phase_c stages all transposed
    blocks into one SBUF tile and writes DRAM with a single contiguous DMA;
  - neighbor rows gathered fp8 via indirect DMA (<=512 idx per call);
  - segment-sum via one-hot matmul on TensorE in fp8 DoubleRow perf mode
    (two 128-slot chunks per instruction, 2x rate), accumulated across both
    halves in one PSUM pass, then scaled by inv-degree (DVE mul);
  - SAGE linears as dense f16 matmuls with fused bias+relu on ScalarE in
    [F, nodes] layout; transpose hT via PE for the next boundary.
"""

import os
import sys

sys.path.insert(0, "/opt/trn_rl_repo")
os.environ.setdefault("MYCRO_LOCAL_CACHE", "1")

import numpy as np

import concourse.bass as bass
import concourse.bacc as bacc
import concourse.mybir as mybir
import concourse.tile as tile
from concourse import bass_utils
from concourse.masks import make_identity

P = 128
F32 = mybir.dt.float32
F16 = mybir.dt.float16
FP8 = mybir.dt.float8e4
I16 = mybir.dt.int16
I32 = mybir.dt.int32
NP_FP8 = mybir.dt.np(FP8)

# >512 idxs per dma_gather hangs real HW (ucode SWDGE ring is fixed-size
# regardless of the build-time carveout), so cap calls at 4 chunks of 128.
DMA_SCRATCH = int(os.environ.get("GNN_DMA_SCRATCH", "16384"))
GC = int(os.environ.get("GNN_GC", "4"))  # max one-hot chunks per dma_gather
NQ = int(os.environ.get("GNN_QUEUES", "4"))  # SWDGE queues to round-robin
TBL16 = os.environ.get("GNN_TBL16", "0") == "1"  # f16 layer-2/3 tables


class Cfg:
    def __init__(self, N=50000, E=800000, in_dim=2, emb=128, hid=256, r=8):
        assert N % r == 0
        self.N, self.E, self.IN_DIM, self.EMB, self.HID, self.R = N, E, in_dim, emb, hid, r
        self.NPC = N // r                      # nodes per core
        self.B = (self.NPC + P - 1) // P       # 128-node blocks per core
        self.NPB = self.B * P                  # padded nodes per core
        self.TBL = r * self.NPB                # 50176 table rows
        self.HA = (r // 2) * self.NPB          # 25088 <= 32768 (int16 ok)
        self.HB = self.TBL - self.HA           # 25088


def host_prep(cfg: Cfg, x, edge_index, W_in, b_in, layers):
    """Build per-core input maps. layers = [(Wl, bl, Wr)] * 3.

    Edges bucketed by (core, dst-block, src-half). Table row for src node
    (core c, local l) = c*NPB + (l%128)*B + l//128  (permuted layout that
    matches the partition-major rows image each core stages). Half A:
    row < HA (cores 0-3); half B: the rest (idx = row - HA). Each
    (block, half) padded to C_A/C_B chunks of 128 slots; pad slots gather
    row 0 with d_rel = -1.
    """
    src = np.asarray(edge_index[0], dtype=np.int64)
    dst = np.asarray(edge_index[1], dtype=np.int64)
    N, E, R, B, NPC, NPB = cfg.N, cfg.E, cfg.R, cfg.B, cfg.NPC, cfg.NPB
    HA = cfg.HA

    deg = np.bincount(dst, minlength=N).astype(np.float64)
    inv = np.where(deg > 0, 1.0 / np.maximum(deg, 1), 0.0).astype(np.float32)

    core = dst // NPC
    local = dst % NPC
    blk = local // P
    drel = (local % P).astype(np.float32)
    group = core * B + blk

    s_core = src // NPC
    s_loc = src % NPC
    srow = s_core * NPB + (s_loc % P) * B + s_loc // P
    regB = (srow >= HA).astype(np.int64)
    srow16 = srow - regB * HA
    grp2 = group * 2 + regB

    cnt2 = np.bincount(grp2, minlength=R * B * 2)
    C_A = int(np.ceil(cnt2[0::2].max() / P))
    C_B = int(np.ceil(cnt2[1::2].max() / P))
    S_A, S_B = C_A * P, C_B * P

    order = np.argsort(grp2, kind="stable")
    offs = np.zeros(R * B * 2 + 1, np.int64)
    np.cumsum(cnt2, out=offs[1:])
    pos = np.arange(E) - offs[grp2[order]]
    gs = group[order]
    regs = regB[order].astype(bool)
    srow_s = srow16[order]
    drel_s = drel[order]

    mA, mB = ~regs, regs
    flatA = gs[mA] * S_A + pos[mA]
    flatB = gs[mB] * S_B + pos[mB]

    idxA = np.zeros(R * B * S_A, np.int16)
    idxA[flatA] = srow_s[mA].astype(np.int16)
    idxB = np.zeros(R * B * S_B, np.int16)
    idxB[flatB] = srow_s[mB].astype(np.int16)

    drelA = np.full(R * B * S_A, -1.0, np.float32)
    drelA[flatA] = drel_s[mA]
    drelB = np.full(R * B * S_B, -1.0, np.float32)
    drelB[flatB] = drel_s[mB]

    # d_rel device layout: [128, B*CT], col b*CT + c, partition = slot % 128
    CT = C_A + C_B
    dA = drelA.reshape(R, B, C_A, P)
    dB = drelB.reshape(R, B, C_B, P)
    drel_dev = np.concatenate([dA, dB], axis=2).transpose(0, 3, 1, 2).reshape(
        R, P, B * CT).astype(np.float16)

    # idx device layout: index i of a half lives at [i % 16, i // 16],
    # replicated across the 8 groups of 16 partitions.
    def idx_dev(arr, S):
        a = arr.reshape(R, B, S // 16, 16).transpose(0, 3, 1, 2).reshape(R, 16, B * (S // 16))
        return np.ascontiguousarray(np.tile(a, (1, 8, 1)))

    idxA_dev = idx_dev(idxA, S_A)
    idxB_dev = idx_dev(idxB, S_B)

    invrow = np.zeros((R, 1, NPB), np.float32)
    invrow[:, 0, :NPC] = inv.reshape(R, NPC)

    # layer-1 node table (host-computed input transform), permuted fp8 rows
    h1 = np.maximum(np.asarray(x, np.float32) @ np.asarray(W_in, np.float32)
                    + np.asarray(b_in, np.float32), 0.0)          # [N, EMB]
    EMB = cfg.EMB
    h1c = h1.reshape(R, NPC, EMB)
    # layer-1 table: fp8 rows padded to 256 bytes (dma_gather needs
    # 256B-multiple rows; 128 fp8 features is only 128B, so pad with zeros).
    t0 = np.zeros((cfg.TBL, 2 * EMB), np.float32)
    l = np.arange(NPC)
    prow = (l % P) * B + l // P
    t0.reshape(R, NPB, 2 * EMB)[:, prow, :EMB] = h1c
    t0 = np.clip(t0, 0.0, 240.0).astype(NP_FP8)

    h1T = np.zeros((R, EMB, NPB), np.float16)
    h1T[:, :, :NPC] = h1c.transpose(0, 2, 1)

    in_maps = []
    for r in range(R):
        m = {
            "h1T": h1T[r],
            "invrow": invrow[r],
            "idxA": idxA_dev[r],
            "idxB": idxB_dev[r],
            "drel": np.ascontiguousarray(drel_dev[r]),
            "t0": t0,   # replicated
        }
        for li, (Wl, bl, Wr) in enumerate(layers, start=1):
            m[f"wl{li}"] = np.asarray(Wl, np.float16)
            m[f"bl{li}"] = np.asarray(bl, np.float32)
            m[f"wr{li}"] = np.asarray(Wr, np.float16)
        in_maps.append(m)
    return in_maps, (C_A, C_B)


def emit(tc: tile.TileContext, outs, ins, cfg: Cfg, C, repeat=1, only=None):
    """only: None for the full pipeline, else a set of phase tags among
    {"a","b","c","ag","a_gather","a_m","a_mm"} for timing ablations."""
    keep = (lambda t: True) if only is None else (lambda t: t in only)
    qrr = [0]
    C_A, C_B = C
    CT = C_A + C_B
    nc = tc.nc
    B, NPB = cfg.B, cfg.NPB
    EMB, HID, R = cfg.EMB, cfg.HID, cfg.R
    groups = [list(range(R))]
    f = F32
    DR = mybir.MatmulPerfMode.DoubleRow
    TDT = F16 if TBL16 else FP8

    # node chunks for the dense-linear phase
    t_chunks = []
    t0c = 0
    while t0c < NPB:
        w = min(512, NPB - t0c)
        t_chunks.append((t0c, w))
        t0c += w

    from contextlib import ExitStack
    ctx = ExitStack()
    const = ctx.enter_context(tc.tile_pool(name="const", bufs=1))
    work = ctx.enter_context(tc.tile_pool(name="work", bufs=int(os.environ.get("GNN_WORK_BUFS", "2"))))
    psA = ctx.enter_context(tc.tile_pool(name="psA", bufs=1, space="PSUM"))
    dram = ctx.enter_context(tc.tile_pool(name="dram", bufs=1, space="DRAM"))

    # ---------------- persistent SBUF ----------------
    h1T = const.tile([P, NPB], F16, name="h1T")
    hT = [const.tile([P, NPB], F16, name=f"hT{k}") for k in range(HID // P)]
    aggT = [const.tile([P, NPB], F16, name=f"aggT{k}") for k in range(HID // P)]
    invrep = const.tile([P, NPB], F16, name="invrep")
    iota_i = const.tile([P, P], I32, name="iota_i")
    iota_h = const.tile([P, P], F16, name="iota_h")
    ident = const.tile([P, P], F16, name="ident")
    idxA_sb = const.tile([P, B * (C_A * P // 16)], I16, name="idxA_sb")
    idxB_sb = const.tile([P, B * (C_B * P // 16)], I16, name="idxB_sb")
    drel_h = const.tile([P, B * CT], F16, name="drel_h")
    drel_f = const.tile([P, B * CT], F32, name="drel_f")
    ones_sb = const.tile([1, P], F16, name="ones_sb")
    rows_sb = const.tile([P, B * HID], TDT, name="rows_sb")

    wl_sb, wr_sb, bl_sb, f_ins = {}, {}, {}, {}
    for li in range(1, 4):
        fin = EMB if li == 1 else HID
        f_ins[li] = fin
        wl_sb[li] = [const.tile([P, HID], F16, name=f"wl{li}_{k}") for k in range(fin // P)]
        wr_sb[li] = [const.tile([P, HID], F16, name=f"wr{li}_{k}") for k in range(fin // P)]
        bl_sb[li] = const.tile([P, HID // P], f, name=f"bl{li}_sb")

    # ---------------- DRAM tables & bounce buffers (per iteration) --------
    def make_tables(it):
        sfx = f"_i{it}" if it else ""
        t = {}
        for li in (1, 2):
            t[li] = (
                dram.tile([cfg.TBL, HID], TDT, addr_space="Shared", name=f"tbl{li}{sfx}"),
                dram.tile([P, B * HID], TDT, name=f"rows{li}{sfx}"),
            )
        return t

    # ---------------- load constants ----------------
    nc.sync.dma_start(h1T[:], ins["h1T"][:])
    nc.sync.dma_start(idxA_sb[:], ins["idxA"][:])
    nc.sync.dma_start(idxB_sb[:], ins["idxB"][:])
    nc.sync.dma_start(drel_h[:], ins["drel"][:])
    nc.vector.tensor_copy(drel_f[:], drel_h[:])
    for li in range(1, 4):
        fin = f_ins[li]
        for k in range(fin // P):
            nc.sync.dma_start(wl_sb[li][k][:], ins[f"wl{li}"][k * P:(k + 1) * P, :])
            nc.sync.dma_start(wr_sb[li][k][:], ins[f"wr{li}"][k * P:(k + 1) * P, :])
        for j in range(HID // P):
            nc.sync.dma_start(bl_sb[li][:, j:j + 1], ins[f"bl{li}"][j * P:(j + 1) * P, None])

    nc.gpsimd.iota(iota_i[:], pattern=[[1, P]], base=0, channel_multiplier=0)
    nc.vector.tensor_copy(iota_h[:], iota_i[:])
    nc.vector.memset(ones_sb[:], 1.0)
    make_identity(nc, ident[:])

    if only is not None:  # ablation mode: make all persistent tiles readable
        for t in hT + aggT:
            nc.vector.memset(t[:], 0.0)
        nc.vector.memset(rows_sb[:], 0.0)

    # invrep[p, n] = inv_deg[n] for all p, via rank-1 matmul broadcast
    for (ts, w) in t_chunks:
        invrow_sb = work.tile([1, 512], F16, tag="invrow", name="invrow_sb")
        nc.gpsimd.dma_start(invrow_sb[:, :w], ins["invrow"][:, ts:ts + w])
        pb = psA.tile([P, 512], f, tag="hlin", bufs=2, name="pb_inv")
        nc.tensor.matmul(pb[:, :w], lhsT=ones_sb[:, :], rhs=invrow_sb[:, :w],
                         start=True, stop=True)
        nc.scalar.copy(invrep[:, ts:ts + w], pb[:, :w])

    def phase_a(fin, tblA, tblB, dt=FP8, fin_pad=None):
        """Gather rows + one-hot segment-sum into aggT (DoubleRow when fp8)."""
        nk = fin // P
        fin_pad = fin_pad or fin
        wIA = C_A * P // 16
        wIB = C_B * P // 16

        gfix = None
        if not keep("a_gather"):  # ablation: stable zero G
            key = (CT, fin_pad, dt)
            if key not in gfix_cache:
                t = const.tile([P, CT, fin_pad], dt, name=f"Gfix{len(gfix_cache)}")
                nc.vector.memset(t[:], 0.0)
                gfix_cache[key] = t
            gfix = gfix_cache[key]
        for b in range(B):
            G = work.tile([P, CT, fin_pad], dt, tag="G", name="G") \
                if gfix is None else gfix
            if keep("a_gather"):
                for (tbl, c0, cN, idx_sb, wI) in (
                        (tblA, 0, C_A, idxA_sb, wIA),
                        (tblB, C_A, C_B, idxB_sb, wIB)):
                    g0 = 0
                    while g0 < cN:
                        g1 = min(g0 + GC, cN)
                        n = (g1 - g0) * P
                        nc.gpsimd.dma_gather(
                            G[:, c0 + g0:c0 + g1, :], tbl,
                            idx_sb[:, b * wI + g0 * 8: b * wI + g1 * 8],
                            n, n, fin_pad, queue_num=qrr[0] % NQ)
                        qrr[0] += 1
                        g0 = g1
            M = work.tile([P, CT, P], dt, tag="M", name="M")
            if keep("a_m"):
                nc.vector.tensor_tensor(
                    out=M[:, :, :],
                    in0=iota_h[:, None, :].broadcast_to([P, CT, P]),
                    in1=drel_h[:, b * CT:(b + 1) * CT, None].broadcast_to([P, CT, P]),
                    op=mybir.AluOpType.is_equal)
            if keep("a_mm"):
                ps = [psA.tile([P, P], f, tag=f"agg{k}", bufs=2,
                               name=f"aggp{k}") for k in range(nk)]
                for k in range(nk):
                    c = 0
                    while c < CT:
                        if dt == FP8 and c + 1 < CT:
                            nc.tensor.matmul(ps[k][:],
                                             lhsT=G[:, c:c + 2, k * P:(k + 1) * P],
                                             rhs=M[:, c:c + 2, :],
                                             start=(c == 0), stop=(c + 2 >= CT),
                                             perf_mode=DR)
                            c += 2
                        else:
                            nc.tensor.matmul(ps[k][:],
                                             lhsT=G[:, c, k * P:(k + 1) * P],
                                             rhs=M[:, c, :],
                                             start=(c == 0), stop=(c + 1 >= CT))
                            c += 1
                bl = slice(b * P, (b + 1) * P)
                for k in range(nk):
                    nc.vector.tensor_mul(aggT[k][:, bl], ps[k][:], invrep[:, bl])

    def phase_b(li):
        fin = f_ins[li]
        nk = fin // P
        hin = [h1T] if li == 1 else hT
        for (ts, w) in t_chunks:
            ph = [psA.tile([P, 512], f, tag="hlin", bufs=2, name=f"ph{li}_{j}")
                  for j in range(HID // P)]
            for j in range(HID // P):
                n_mm = 2 * nk
                i_mm = 0
                for k in range(nk):
                    nc.tensor.matmul(ph[j][:, :w], lhsT=wl_sb[li][k][:, j * P:(j + 1) * P],
                                     rhs=aggT[k][:, ts:ts + w],
                                     start=(i_mm == 0), stop=(i_mm == n_mm - 1))
                    i_mm += 1
                for k in range(nk):
                    nc.tensor.matmul(ph[j][:, :w], lhsT=wr_sb[li][k][:, j * P:(j + 1) * P],
                                     rhs=hin[k][:, ts:ts + w],
                                     start=(i_mm == 0), stop=(i_mm == n_mm - 1))
                    i_mm += 1
            for j in range(HID // P):
                nc.scalar.activation(hT[j][:, ts:ts + w], ph[j][:, :w],
                                     mybir.ActivationFunctionType.Relu,
                                     bias=bl_sb[li][:, j:j + 1])

    def phase_c(tbl=None, rows=None, rows_out=None):
        """transpose hT -> staged rows; one DMA + one AllGather per boundary.
        If rows_out is given, write f32 node-major rows there (final out)."""
        if not keep("c"):
            return
        if rows_out is not None:
            for b in range(B):
                stage = work.tile([P, HID], F32, tag="stage", name="stage")
                for j in range(HID // P):
                    trp = psA.tile([P, P], F16, tag="trp", bufs=2, name="trp_c")
                    nc.tensor.transpose(trp[:], hT[j][:, b * P:(b + 1) * P], ident[:])
                    nc.vector.tensor_copy(stage[:, j * P:(j + 1) * P], trp[:])
                nc.sync.dma_start(rows_out[b * P:(b + 1) * P, :], stage[:])
            return
        for b in range(B):
            for j in range(HID // P):
                trp = psA.tile([P, P], F16, tag="trp", bufs=2, name="trp_c")
                nc.tensor.transpose(trp[:], hT[j][:, b * P:(b + 1) * P], ident[:])
                nc.vector.tensor_copy(
                    rows_sb[:, b * HID + j * P: b * HID + (j + 1) * P], trp[:])
        nc.sync.dma_start(rows[:, :], rows_sb[:, :])
        if keep("ag"):
            nc.gpsimd.collective_compute(
                "AllGather", mybir.AluOpType.bypass, replica_groups=groups,
                ins=[rows.opt()], outs=[tbl.opt()])

    def pipeline(it):
        t = make_tables(it)
        if keep("a"):
            phase_a(EMB, ins["t0"][:cfg.HA, :], ins["t0"][cfg.HA:, :],
                    dt=FP8, fin_pad=2 * EMB)
        if keep("b"):
            phase_b(1)
        phase_c(*t[1])
        if keep("a"):
            phase_a(HID, t[1][0][:cfg.HA, :], t[1][0][cfg.HA:, :], dt=TDT)
        if keep("b"):
            phase_b(2)
        phase_c(*t[2])
        if keep("a"):
            phase_a(HID, t[2][0][:cfg.HA, :], t[2][0][cfg.HA:, :], dt=TDT)
        if keep("b"):
            phase_b(3)
        phase_c(rows_out=outs["h_out"])

    gfix_cache = {}
    for it in range(repeat):
        pipeline(it)
    ctx.close()


def build_program(cfg: Cfg, C, repeat=1, only=None):
    C_A, C_B = C
    CT = C_A + C_B
    nc = bacc.Bacc("TRN2", target_bir_lowering=False, debug=False,
                   enable_asserts=True, num_devices=cfg.R,
                   dynamic_dma_scratch_size=DMA_SCRATCH,
                   num_swdge_queues=NQ)
    ins = {
        "h1T": nc.dram_tensor("h1T", [cfg.EMB, cfg.NPB], F16, kind="ExternalInput").ap(),
        "invrow": nc.dram_tensor("invrow", [1, cfg.NPB], F32, kind="ExternalInput").ap(),
        "idxA": nc.dram_tensor("idxA", [P, cfg.B * (C_A * P // 16)], I16, kind="ExternalInput").ap(),
        "idxB": nc.dram_tensor("idxB", [P, cfg.B * (C_B * P // 16)], I16, kind="ExternalInput").ap(),
        "drel": nc.dram_tensor("drel", [P, cfg.B * CT], F16, kind="ExternalInput").ap(),
        "t0": nc.dram_tensor("t0", [cfg.TBL, 2 * cfg.EMB], FP8, kind="ExternalInput").ap(),
    }
    for li in range(1, 4):
        fin = cfg.EMB if li == 1 else cfg.HID
        ins[f"wl{li}"] = nc.dram_tensor(f"wl{li}", [fin, cfg.HID], F16, kind="ExternalInput").ap()
        ins[f"bl{li}"] = nc.dram_tensor(f"bl{li}", [cfg.HID], F32, kind="ExternalInput").ap()
        ins[f"wr{li}"] = nc.dram_tensor(f"wr{li}", [fin, cfg.HID], F16, kind="ExternalInput").ap()
    outs = {
        "h_out": nc.dram_tensor("h_out", [cfg.NPB, cfg.HID], F32, kind="ExternalOutput").ap(),
    }
    with tile.TileContext(nc) as tc:
        emit(tc, outs, ins, cfg, C, repeat=repeat, only=only)
    nc.compile()
    return nc


def make_runner(nc, in_maps, n_cores):
    """Build a pinned-input PJRT runner: inputs are device_put once, outputs
    are donated back as the next call's (ignored) output buffers, so warm
    calls measure execution + dispatch only."""
    import jax
    from jax.sharding import Mesh, PartitionSpec, NamedSharding
    from jax.experimental.shard_map import shard_map
    from concourse import bass2jax, mybir as mb
    bass2jax.install_neuronx_cc_hook()

    partition_name = nc.partition_id_tensor.name if nc.partition_id_tensor else None
    in_names, out_names, out_avals, zero_outs = [], [], [], []
    for alloc in nc.m.functions[0].allocations:
        if not isinstance(alloc, mb.MemoryLocationSet):
            continue
        name = alloc.memorylocations[0].name
        if alloc.kind == "ExternalInput":
            if name != partition_name:
                in_names.append(name)
        elif alloc.kind == "ExternalOutput":
            shape = tuple(alloc.tensor_shape)
            dtype = mb.dt.np(alloc.dtype)
            out_names.append(name)
            out_avals.append(jax.core.ShapedArray(shape, dtype))
            zero_outs.append(np.zeros(shape, dtype))
    n_params = len(in_names)
    n_outs = len(out_avals)
    all_in_names = list(in_names) + list(out_names)
    if partition_name is not None:
        all_in_names.append(partition_name)
    donate = tuple(range(n_params, n_params + n_outs))

    def _body(*args):
        operands = list(args)
        if partition_name is not None:
            operands.append(bass2jax.partition_id_tensor())
        outs = bass2jax._bass_exec_p.bind(
            *operands,
            out_avals=tuple(out_avals),
            in_names=tuple(all_in_names),
            out_names=tuple(out_names),
            lowering_input_output_aliases=(),
            sim_require_finite=True,
            sim_require_nnan=True,
            nc=nc,
        )
        return tuple(outs)

    devices = jax.devices()[:n_cores]
    mesh = Mesh(np.asarray(devices), ("core",))
    sharded = jax.jit(
        shard_map(_body, mesh=mesh,
                  in_specs=(PartitionSpec("core"),) * (n_params + n_outs),
                  out_specs=(PartitionSpec("core"),) * n_outs,
                  check_rep=False),
        donate_argnums=donate, keep_unused=True)
    sh = NamedSharding(mesh, PartitionSpec("core"))
    concat_in = [np.concatenate([np.asarray(m[nm]) for m in in_maps], axis=0)
                 for nm in in_names]
    d_in = [jax.device_put(a, sh) for a in concat_in]
    state = {"donate": [jax.device_put(
        np.zeros((n_cores * z.shape[0], *z.shape[1:]), z.dtype), sh) for z in zero_outs]}

    def run():
        outs = sharded(*d_in, *state["donate"])
        jax.block_until_ready(outs)
        state["donate"] = list(outs)
        return outs

    def results():
        outs = state["donate"]
        return [{nm: np.asarray(outs[i]).reshape(n_cores, *out_avals[i].shape)[c]
                 for i, nm in enumerate(out_names)} for c in range(n_cores)]

    return run, results


def kernel(**inputs) -> np.ndarray:
    cfg = Cfg()
    layers = [(inputs["Wl1"], inputs["bl1"], inputs["Wr1"]),
              (inputs["Wl2"], inputs["bl2"], inputs["Wr2"]),
              (inputs["Wl3"], inputs["bl3"], inputs["Wr3"])]
    in_maps, C = host_prep(cfg, inputs["x"], inputs["edge_index"],
                           inputs["W_in"], inputs["b_in"], layers)
    nc = build_program(cfg, C)
    # the NTFF trace hook (antenv.axon_hooks) is absent in this container;
    # make sure run_bass_kernel_spmd never takes the trace path.
    os.environ["BASS_NEVER_TRACE"] = "1"
    res = bass_utils.run_bass_kernel_spmd(
        nc, in_maps, core_ids=list(range(cfg.R)), trace=False)
    if res.exec_time_ns is not None:
        print(f"HW exec time: {res.exec_time_ns} ns")
    out = np.concatenate(
        [res.results[r]["h_out"][:cfg.NPC] for r in range(cfg.R)], axis=0)
    return out.astype(np.float32)


if __name__ == "__main__":
    # smoke: build only
    cfg = Cfg()
    rng = np.random.default_rng(0)
    ei = rng.integers(0, cfg.N, size=(2, cfg.E), dtype=np.int64)
    x = rng.standard_normal((cfg.N, cfg.IN_DIM), dtype=np.float32)
    layers = [(rng.standard_normal((cfg.EMB, cfg.HID), dtype=np.float32) * 0.1,
               np.zeros(cfg.HID, np.float32),
               rng.standard_normal((cfg.EMB, cfg.HID), dtype=np.float32) * 0.1)]
    layers += [(rng.standard_normal((cfg.HID, cfg.HID), dtype=np.float32) * 0.1,
                np.zeros(cfg.HID, np.float32),
                rng.standard_normal((cfg.HID, cfg.HID), dtype=np.float32) * 0.1) for _ in range(2)]
    in_maps, C = host_prep(cfg, x, ei, rng.standard_normal((cfg.IN_DIM, cfg.EMB), dtype=np.float32),
                           np.zeros(cfg.EMB, np.float32), layers)
    print("C =", C)
    nc = build_program(cfg, C)
    print("built ok; instructions:",
          sum(len(bb.instructions) for fn in nc.m.functions for bb in fn.blocks))
